# revision 71
# baseline (speedup 1.0000x reference)
"""Causal self-attention with RoPE on 8 Trainium2 NeuronCores.

Problem (hardcoded): x (4, 2048, 2048) f32, w_attn (2048, 6144),
w_proj (2048, 2048), rope_cos/rope_sin (2048, 64), 16 heads, hd=128.

Sharding: 8 cores = 4 batches x 2 head-groups (8 heads each).  Each core
computes qkv projection for its heads, RoPE, causal attention, and a
partial output projection (its head-group's rows of w_proj).  The host
sums the two partials per batch (the "all-reduce after c_proj") and
transposes back, since the device kernel works fully transposed.

v3 design notes (PE floor for this decomposition is ~560us/core):
  - bf16 operands everywhere (f32 PSUM accumulation).  Halves SBUF and
    HBM traffic so x (8MB) and v (4MB) stay SBUF-resident for the whole
    kernel; q/k round-trip through DRAM at 0.5MB/head.
  - Attention is locally ACT(exp)-bound: per head the exp+norm chain is
    ~29us of ACT vs ~15us of PE.  Every head therefore gets ACT-free PE
    filler interleaved between its Q-chunks: the v projection (heads
    0-4), the DEFERRED q/k projection of heads 6-7 (heads 2-6), and the
    whole output projection (head 7's tail).  The PE never waits for
    the exp backlog to drain.
  - Softmax denominators: exp tiles are accumulated on Pool/DVE into a
    per-chunk f32r acc tile; ONE ones[128,128] matmul row-sums AND
    broadcasts it, then 1/den = Exp(-Ln(den)) on ACT (Ln/Exp share one
    activation table set; custom-DVE approx reciprocal doesn't compile
    on this walrus build and exact DVE reciprocal costs 3.3us/tile).
  - Normalization of o^T is deferred one Q-chunk so the PE never waits
    on the exp/reciprocal chain.
  - No scalar-engine DMA issues during attention (they would stall the
    saturated ACT stream): phase-B-era DMAs ride the sync/gpsimd queues.
"""

import sys

sys.path.insert(0, "/opt/trn_rl_repo")

import numpy as np
import ml_dtypes

import concourse.bass as bass
import concourse.mybir as mybir
import concourse.tile as tile

F32 = mybir.dt.float32
F32R = mybir.dt.float32r
BF16 = mybir.dt.bfloat16
P = 128


# --------------------------------------------------------------------------
# This container's walrus build rejects any instruction carrying more than
# one sem wait.  Split extras onto NoOps inserted before the instruction on
# the same engine (per-engine program order makes the waits complete first).
def _split_multi_waits(nc):
    n = 0
    for fn in nc.m.functions:
        for bb in fn.blocks:
            out = []
            changed = False
            for inst in bb.instructions:
                si = inst.sync_info
                waits = list(si.on_wait or []) if si is not None else []
                if len(waits) > 1:
                    changed = True
                    n += 1
                    for w in waits[:-1]:
                        nop = mybir.InstNoOp(
                            name=nc.get_next_instruction_name(),
                            engine=inst.engine,
                            ins=[],
                            outs=[],
                            sync_info=mybir.SyncInfo(on_wait=[w], on_update=[]),
                        )
                        try:
                            nc.register_instruction(nop, overwrite=True)
                        except Exception:
                            pass
                        out.append(nop)
                    inst.sync_info = mybir.SyncInfo(
                        on_wait=[waits[-1]], on_update=list(si.on_update or [])
                    )
                out.append(inst)
            if changed:
                bb.instructions = out
    return n


def build_attention_core(T=2048, C=2048, G=8):
    """One core's program.  T tokens, C model dim, G heads in this core's
    group (hd=128 each).  Returns the Bass object."""
    KO = C // P           # 16 contraction tiles over model dim
    NQ = T // 512         # 4 q chunks
    QW = 512
    JPQ = QW // P         # 4 j tiles per q chunk width
    NJ = T // P           # 16 total j tiles
    NTB = NJ              # t blocks for v
    VN = 512              # v column chunk (4 heads)
    NV = (G * P) // VN    # 2 v chunks
    NM_A = 12             # q/k head-tiles computed in phase A (heads 0-5)

    nc = bass.Bass()
    xt = nc.dram_tensor("xt", [P, KO, T], BF16, kind="ExternalInput")
    # m interleaved: 2h = q head h, 2h+1 = k head h (RoPE pair-permuted)
    wqk = nc.dram_tensor("wqk", [2 * G, P, KO, P], BF16, kind="ExternalInput")
    wv = nc.dram_tensor("wv", [NV, P, KO, VN], BF16, kind="ExternalInput")
    wp = nc.dram_tensor("wp", [KO, P, G, P], BF16, kind="ExternalInput")
    # cosd = [cos; cos], sind = [-sin; +sin]  (rope = raw*cosd + swapped*sind)
    cosp = nc.dram_tensor("cosp", [P, T], BF16, kind="ExternalInput")
    sinp = nc.dram_tensor("sinp", [P, T], BF16, kind="ExternalInput")
    maskt = nc.dram_tensor("maskt", [P, P], BF16, kind="ExternalInput")
    onesd = nc.dram_tensor("onesd", [P, P], BF16, kind="ExternalInput")
    outT = nc.dram_tensor("outT", [C, T], BF16, kind="ExternalOutput")

    scale = 1.0 / np.sqrt(128.0)

    with tile.TileContext(nc) as tc:
        with (
            tc.tile_pool(name="dram", bufs=1, space="DRAM") as dram,
            tc.tile_pool(name="const", bufs=1) as cpool,
            tc.tile_pool(name="xres", bufs=1) as x_pool,
            tc.tile_pool(name="vall", bufs=1) as va_pool,
            tc.tile_pool(name="oacc", bufs=G) as oacc_pool,
            tc.tile_pool(name="wv", bufs=1) as wv_pool,
        ):
            qkd = dram.tile([2 * G, P, T], BF16)

            # Start sequence: first head's weights, then x in kc-major
            # slices so the m=0 accumulation can chase the arrivals.
            x_s = x_pool.tile([P, KO, T], BF16, tag="x")
            wqk0 = None  # emitted inside phase A loop; queue order set here
            xsl = [(0, 1), (1, 2), (2, 4), (4, 6), (6, 8), (8, 10),
                   (10, 12), (12, 14), (14, 16)]

            # v resident for phases A+B:
            # v_all[ti, to, hh*128+d] = v[to*128+ti, head hh, d]
            v_all = va_pool.tile([P, NJ, G * P], BF16, tag="vall")

            cos_s = cpool.tile([P, T], BF16)
            sin_s = cpool.tile([P, T], BF16)
            mask_s = cpool.tile([P, P], BF16)
            ones_s = cpool.tile([P, P], BF16)

            # wv0 DMA issues at the very top of the gpsimd queue: the v0
            # blocks open phase B and must not wait behind phase A's swaps
            wv_s = [None, None]
            wv_s[0] = wv_pool.tile([P, KO, VN], BF16, tag="wv", name="wv0")

            # ------------- Phase A: q,k heads 0-5 + RoPE -------------
            with (
                tc.tile_pool(name="wqk", bufs=3) as wqk_pool,
                tc.tile_pool(name="qkraw", bufs=2) as qkraw_pool,
                tc.tile_pool(name="ropesw", bufs=2) as rsw_pool,
                tc.tile_pool(name="roped", bufs=2) as roped_pool,
                tc.tile_pool(name="psA", bufs=2, space="PSUM") as psA,
            ):
                def emit_rope_A(m, pss):
                    # RoPE: rope = raw*[cos;cos] + swap(raw)*[-sin;+sin]
                    raw = qkraw_pool.tile([P, T], BF16, tag="qkraw")
                    nc.scalar.copy(raw[:], pss[:])
                    sw = rsw_pool.tile([P, T], BF16, tag="rsw")
                    nc.gpsimd.dma_start(sw[0:64, :], raw[64:128, :])
                    nc.gpsimd.dma_start(sw[64:128, :], raw[0:64, :])
                    rop = roped_pool.tile([P, T], BF16, tag="roped")
                    with nc.allow_low_precision(reason="bf16 rope"):
                        nc.vector.tensor_mul(rop[:], raw[:], cos_s[:])
                        nc.vector.tensor_mul(sw[:], sw[:], sin_s[:])
                        nc.vector.tensor_add(rop[:], rop[:], sw[:])
                    nc.sync.dma_start(qkd[m], rop[:])

                # m=0,1 run a fused kc loop: one head-tile alone consumes x
                # slower than DMA delivers it, so the PE would stall chasing
                # arrivals; two tiles per kc keep it fed (and fill all 8
                # PSUM banks)
                w01 = []
                for m in range(2):
                    w_s = wqk_pool.tile([P, KO, P], BF16, tag="wqk",
                                        name=f"w{m}")
                    nc.scalar.dma_start(w_s[:, 0:2, :], wqk[m][:, 0:2, :])
                    w01.append(w_s)
                nc.sync.dma_start(x_s[:, 0:1, :], xt[:, 0:1, :])
                nc.scalar.dma_start(w01[0][:, 2:6, :], wqk[0][:, 2:6, :])
                nc.scalar.dma_start(w01[1][:, 2:6, :], wqk[1][:, 2:6, :])
                nc.sync.dma_start(x_s[:, 1:2, :], xt[:, 1:2, :])
                nc.scalar.dma_start(w01[0][:, 6:, :], wqk[0][:, 6:, :])
                nc.scalar.dma_start(w01[1][:, 6:, :], wqk[1][:, 6:, :])
                for a, b in xsl[2:]:
                    nc.sync.dma_start(x_s[:, a:b, :], xt[:, a:b, :])
                # consts/wv0 ride the SYNC ring behind all of x: per-ring
                # FIFO keeps them off the HBM until the kc chase is done
                # (cos/sin land just before m=0's RoPE needs them)
                nc.sync.dma_start(cos_s[:], cosp[:])
                nc.sync.dma_start(sin_s[:], sinp[:])
                nc.sync.dma_start(mask_s[:], maskt[:])
                nc.sync.dma_start(ones_s[:], onesd[:])
                nc.sync.dma_start(wv_s[0][:], wv[0])
                pss01 = [psA.tile([P, T], F32, tag="pqk", name=f"pqk{m}")
                         for m in range(2)]
                for kc in range(KO):
                    for m in range(2):
                        for i in range(NQ):
                            nc.tensor.matmul(
                                pss01[m][:, i * QW : (i + 1) * QW],
                                w01[m][:, kc, :],
                                x_s[:, kc, i * QW : (i + 1) * QW],
                                start=(kc == 0),
                                stop=(kc == KO - 1),
                                skip_group_check=True,
                            )
                emit_rope_A(0, pss01[0])
                emit_rope_A(1, pss01[1])

                for m in range(2, NM_A):
                    w_s = wqk_pool.tile([P, KO, P], BF16, tag="wqk",
                                        name=f"w{m}")
                    nc.scalar.dma_start(w_s[:], wqk[m])
                    pss = psA.tile([P, T], F32, tag="pqk", name=f"pqk{m}")
                    for kc in range(KO):
                        for i in range(NQ):
                            nc.tensor.matmul(
                                pss[:, i * QW : (i + 1) * QW],
                                w_s[:, kc, :],
                                x_s[:, kc, i * QW : (i + 1) * QW],
                                start=(kc == 0),
                                stop=(kc == KO - 1),
                                skip_group_check=True,
                            )
                    emit_rope_A(m, pss)

            # ------- Phase B/C: v, attention, deferred q/k, out proj -------
            with (
                tc.tile_pool(name="qh", bufs=2) as q_pool,
                tc.tile_pool(name="kh", bufs=2) as k_pool,
                tc.tile_pool(name="pt", bufs=6) as pt_pool,
                tc.tile_pool(name="acc", bufs=2) as acc_pool,
                tc.tile_pool(name="accb", bufs=1) as accb_pool,
                tc.tile_pool(name="rinv", bufs=2) as rinv_pool,
                tc.tile_pool(name="dq_w", bufs=2) as dqw_pool,
                tc.tile_pool(name="dq_t", bufs=1) as dqt_pool,
                tc.tile_pool(name="wp", bufs=4) as wp_pool,
                tc.tile_pool(name="csb", bufs=2) as csb_pool,
                tc.tile_pool(name="psS", bufs=4, space="PSUM") as psS,
                tc.tile_pool(name="psO", bufs=2, space="PSUM") as psO,
                tc.tile_pool(name="psM", bufs=2, space="PSUM") as psM,
            ):
                def emit_v_block(n2, tb):
                    psv = psM.tile([P, VN], F32, tag="psm")
                    for kc in range(KO):
                        nc.tensor.matmul(
                            psv[:],
                            x_s[:, kc, tb * P : (tb + 1) * P],
                            wv_s[n2][:, kc, :],
                            start=(kc == 0),
                            stop=(kc == KO - 1),
                        )
                    # Pool can't read PSUM on trn2; DVE takes this copy
                    with nc.allow_low_precision(reason="v is bf16"):
                        nc.vector.tensor_copy(
                            v_all[:, tb, n2 * VN : (n2 + 1) * VN], psv[:]
                        )

                def emit_wv1_load():
                    wv_s[1] = wv_pool.tile([P, KO, VN], BF16, tag="wv",
                                           name="wv1")
                    nc.gpsimd.dma_start(wv_s[1][:], wv[1])

                # deferred q/k head-tile, in 512-wide chunks ([128,512]
                # PSUM ring; phase A's 4-bank tiles don't fit here).  m=15
                # (k of head 7) writes straight into head 7's SBUF k tile:
                # a DRAM round-trip would race the ring-buffer prefetch.
                def emit_dqk_load(m):
                    dw = dqw_pool.tile([P, KO, P], BF16, tag="dw",
                                       name=f"dw{m}")
                    nc.sync.dma_start(dw[:], wqk[m])
                    emit_dqk_chunk.w[m] = dw

                def emit_dqk_chunk(m, c):
                    sl = slice(c * QW, (c + 1) * QW)
                    if m not in emit_dqk_chunk.w:
                        emit_dqk_load(m)
                    dw = emit_dqk_chunk.w[m]
                    psq = psM.tile([P, QW], F32, tag="psm")
                    for kc in range(KO):
                        nc.tensor.matmul(
                            psq[:], dw[:, kc, :], x_s[:, kc, sl],
                            start=(kc == 0), stop=(kc == KO - 1),
                            skip_group_check=True,
                        )
                    raw = dqt_pool.tile([P, QW], BF16, tag="draw")
                    with nc.allow_low_precision(reason="bf16 rope"):
                        nc.vector.tensor_copy(raw[:], psq[:])
                    sw = dqt_pool.tile([P, QW], BF16, tag="dsw")
                    nc.gpsimd.dma_start(sw[0:64, :], raw[64:128, :])
                    nc.gpsimd.dma_start(sw[64:128, :], raw[0:64, :])
                    if m == 15:
                        if 7 not in kts:
                            kts[7] = k_pool.tile([P, T], BF16, tag="k",
                                                 name="k7")
                        rop_out = kts[7][:, sl]
                    else:
                        rop_out = None
                    rop = dqt_pool.tile([P, QW], BF16, tag="drop")
                    with nc.allow_low_precision(reason="bf16 rope"):
                        nc.vector.tensor_mul(rop[:], raw[:], cos_s[:, sl])
                        nc.vector.tensor_mul(sw[:], sw[:], sin_s[:, sl])
                        if rop_out is not None:
                            nc.vector.tensor_add(rop_out, rop[:], sw[:])
                        else:
                            nc.vector.tensor_add(rop[:], rop[:], sw[:])
                            nc.sync.dma_start(qkd[m][:, sl], rop[:])

                emit_dqk_chunk.w = {}

                # phase C as fine-grained per-m pieces so they can weave
                # into head 7's J-loops; wp streams through a 4-slot ring,
                # loaded one quad ahead of its consumers
                wp_tiles = {}

                def emit_wp_loads(t, mg):
                    for m in range(mg * 4, mg * 4 + 4):
                        wpt = wp_pool.tile([P, G, P], BF16, tag="wp",
                                           name=f"wp{t}_{m}")
                        nc.sync.dma_start(wpt[:], wp[m])
                        wp_tiles[(t, m)] = wpt

                def emit_C_m(t, m):
                    psc = psM.tile([P, QW], F32, tag="psm")
                    wpt = wp_tiles.pop((t, m))
                    for hh in range(G):
                        nc.tensor.matmul(
                            psc[:],
                            wpt[:, hh, :],
                            oTs[hh][:, t * QW : (t + 1) * QW],
                            start=(hh == 0),
                            stop=(hh == G - 1),
                        )
                    csb = csb_pool.tile([P, QW], BF16, tag="csb")
                    # alternate engines: DVE alone backs up during the
                    # C-era, but ACT can't take them all either (C pieces
                    # weave into head 7's exp-saturated chunks)
                    if m % 2 == 0:
                        with nc.allow_low_precision(reason="bf16 out"):
                            nc.vector.tensor_copy(csb[:], psc[:])
                    else:
                        nc.scalar.copy(csb[:], psc[:])
                    nc.sync.dma_start(
                        outT[m * P : (m + 1) * P, t * QW : (t + 1) * QW],
                        csb[:],
                    )

                def add_C_t(h, q, t):
                    # per t-chunk: 16 m pieces with the NEXT quad's wp load
                    # emitted before each quad's pieces (ring stays 1 ahead)
                    for mg in range(4):
                        if mg + 1 < 4:
                            add(h, q, emit_wp_loads, t, mg + 1)
                        elif t + 1 < NQ:
                            add(h, q, emit_wp_loads, t + 1, 0)
                        for m in range(mg * 4, mg * 4 + 4):
                            add(h, q, emit_C_m, t, m)

                # ---- PE filler schedule: slot (h, Q) -> list of thunks ----
                # v0 = v heads 0-3 (16 blocks), v1 = v heads 4-7 (16),
                # dqk m=12..15 = q/k heads 6-7, C = output projection.
                # Constraints: v0 tb in [4Q,4Q+3] before (0..3, Q); v1 tb
                # likewise before (4, Q); dqk m before head m//2; C t-chunk
                # after head 7's Q=t norm (deferred: emitted during Q=t+1).
                fill = {}

                def add(h, q, fn, *a):
                    fill.setdefault((h, q), []).append((fn, a))

                for tb in range(4, 8):
                    add(0, 0, emit_v_block, 0, tb)
                for tb in range(8, 12):
                    add(0, 1, emit_v_block, 0, tb)
                for tb in range(12, 16):
                    add(0, 2, emit_v_block, 0, tb)
                add(0, 3, emit_wv1_load)
                for tb in range(0, 3):
                    add(1, tb + 1, emit_v_block, 1, tb)
                for tb in range(3, 7):
                    add(2, tb - 3, emit_v_block, 1, tb)
                add(1, 3, emit_dqk_load, 12)
                add(2, 3, emit_dqk_load, 13)
                add(4, 3, emit_dqk_load, 14)
                add(5, 3, emit_dqk_load, 15)
                for i, c in enumerate(range(4)):     # m=12 in head 2/3
                    add(2 if i < 2 else 3, i % 2 * 2, emit_dqk_chunk, 12, c)
                for tb in range(7, 11):
                    add(3, tb - 7, emit_v_block, 1, tb)
                for c in range(4):                   # m=13 in head 3/4
                    add(3 if c < 2 else 4, c % 2, emit_dqk_chunk, 13, c)
                # (4,Q) consumes v1 tb<=4Q+3, so tb12-15 must precede (4,3)
                for tb, q in [(11, 0), (12, 0), (13, 1), (14, 1), (15, 2)]:
                    add(4, q, emit_v_block, 1, tb)
                for c in range(4):                   # m=14 in head 5
                    add(5, c, emit_dqk_chunk, 14, c)
                for c in range(4):                   # m=15 in head 6
                    add(6, c, emit_dqk_chunk, 15, c)
                add(6, 3, emit_wp_loads, 0, 0)
                add_C_t(7, 1, 0)
                add_C_t(7, 2, 1)
                add_C_t(7, 3, 2)

                # v0 tb0-3 must precede head 0 entirely
                for tb in range(4):
                    emit_v_block(0, tb)

                # q/k prefetch for heads 0,1 (heads 2+ prefetched inside the
                # loop).  Head 7's k never touches DRAM (m=15 writes its
                # SBUF tile directly), and its q prefetch is emitted after
                # m=14's qkd writes, so program order keeps RAW intact.
                qts, kts = {}, {}

                def prefetch_q(hp):
                    qts[hp] = q_pool.tile([P, T], BF16, tag="q",
                                          name=f"q{hp}")
                    nc.sync.dma_start(qts[hp][:], qkd[2 * hp])

                def prefetch_k(hp):
                    kts[hp] = k_pool.tile([P, T], BF16, tag="k",
                                          name=f"k{hp}")
                    nc.sync.dma_start(kts[hp][:], qkd[2 * hp + 1])

                def prefetch_qk(hp):
                    prefetch_q(hp)
                    if hp != 7:
                        prefetch_k(hp)

                prefetch_qk(0)
                prefetch_qk(1)

                def emit_attn_J(h, Q, J, qT, kT, acc_a, acc_b):
                    k_d = J - JPQ * Q  # diag idx if >= 0
                    co = max(k_d, 0) * P
                    pss = psS.tile([P, QW], F32, tag="pss")
                    nc.tensor.matmul(
                        pss[:, co:],
                        kT[:, J * P : (J + 1) * P],
                        qT[:, Q * QW + co : (Q + 1) * QW],
                        start=True,
                        stop=True,
                        skip_group_check=True,
                    )
                    pT = pt_pool.tile([P, QW], BF16, tag="pt")
                    nc.scalar.activation(
                        pT[:, co:], pss[:, co:],
                        mybir.ActivationFunctionType.Exp,
                        scale=scale,
                    )
                    if k_d >= 0:
                        with nc.allow_low_precision(reason="bf16 attn"):
                            nc.vector.tensor_mul(
                                pT[:, co : co + P],
                                pT[:, co : co + P],
                                mask_s[:],
                            )
                    # den chain split across DVE (even J) and Pool (odd J):
                    # either engine alone can't keep up with the exp cadence
                    # and a lagging chain holds PSUM ring slots, stalling PE
                    with nc.allow_low_precision(reason="f32 bits"):
                        if J == 0:
                            nc.vector.tensor_copy(acc_a[:], pT[:])
                        elif J == 1:
                            nc.gpsimd.tensor_copy(
                                acc_b[:, co:], pT[:, co:]
                            )
                        elif J % 2 == 0:
                            nc.vector.tensor_add(
                                acc_a[:, co:], acc_a[:, co:], pT[:, co:]
                            )
                        else:
                            nc.gpsimd.tensor_add(
                                acc_b[:, co:], acc_b[:, co:], pT[:, co:]
                            )
                    return co, pT

                def emit_chunk(h, Q, qT, kT, oT, den_pend, norm_pend,
                               emit_den, emit_norm):
                    jmax = JPQ * (Q + 1) - 1  # inclusive
                    # at head 7 the previous chunk's den reduction + norm
                    # run FIRST so the woven phase C pieces are legal; for
                    # other heads they stay post-loop (at the top, the
                    # psden matmul can stall the whole J-loop on the
                    # combine when the slot's fillers are small)
                    if h == 7:
                        emit_den()
                        if norm_pend:
                            emit_norm()
                    pso = psO.tile([P, QW], F32, tag="pso")
                    acc_a = acc_pool.tile([P, QW], BF16, tag="acca")
                    acc_b = accb_pool.tile([P, QW], BF16, tag="accb")
                    # PV(J) consumes exp(S(J)) from ACT; emit it LOOK
                    # S-matmuls later so PE never stalls on ACT.  The exp
                    # cadence (~535ns) exceeds the PE's per-J work (~426ns),
                    # so ACT-independent filler is woven in every few J's
                    # to keep the PE fed while ACT catches up.
                    LOOK = 3
                    fillers = fill.pop((h, Q), [])
                    pend = []
                    for J in range(jmax + 1 + LOOK):
                        if J <= jmax:
                            pend.append(
                                (J,)
                                + emit_attn_J(h, Q, J, qT, kT, acc_a, acc_b)
                            )
                        if J >= LOOK:
                            Jp, cop, pTp = pend.pop(0)
                            nc.tensor.matmul(
                                pso[:, cop:],
                                v_all[:, Jp, h * P : (h + 1) * P],
                                pTp[:, cop:],
                                start=(Jp == 0),
                                stop=(Jp == jmax),
                                skip_group_check=True,
                            )
                        if J % 4 == 3 and fillers:
                            fn, a = fillers.pop(0)
                            fn(*a)
                    # raw o^T out of PSUM (DVE: ACT is exp-saturated here)
                    with nc.allow_low_precision(reason="bf16 out"):
                        nc.vector.tensor_copy(
                            oT[:, Q * QW : (Q + 1) * QW], pso[:]
                        )
                    # den combine runs on DVE NOW (right behind this chunk's
                    # own adds, so it drains ASAP), but the dependent psden
                    # matmul is deferred one chunk (emit_den above) so the
                    # in-order PE never waits on the chain.
                    # acc_b is initialized at J=1, whose co is 128 for Q=0
                    # (columns below have no odd-J contributions)
                    cb = P if Q == 0 else 0
                    with nc.allow_low_precision(reason="f32 bits"):
                        nc.vector.tensor_add(
                            acc_a[:, cb:], acc_a[:, cb:], acc_b[:, cb:]
                        )
                    if h != 7:
                        # den(Q-1): its combine drained during this J-loop
                        emit_den()
                        if norm_pend:
                            emit_norm()
                    den_pend.append((Q, acc_a))
                    # remaining PE filler for this slot
                    for fn, a in fillers:
                        fn(*a)

                oTs = []

                def process_head(h):
                    qT = qts.pop(h)
                    kT = kts.pop(h)
                    oT = oacc_pool.tile([P, T], BF16, tag="oacc",
                                        name=f"oT{h}")
                    oTs.append(oT)

                    # deferred den-reduction / normalization state
                    den_pend = []
                    norm_pend = []

                    def emit_den(den_pend=den_pend, norm_pend=norm_pend):
                        if not den_pend:
                            return
                        Qp, acc_a = den_pend.pop(0)
                        psden = psM.tile([P, QW], F32, tag="psm")
                        nc.tensor.matmul(
                            psden[:], ones_s[:], acc_a[:],
                            start=True, stop=True, skip_group_check=True,
                        )
                        nc.scalar.activation(
                            psden[:], psden[:],
                            mybir.ActivationFunctionType.Ln,
                        )
                        rinv = rinv_pool.tile([P, QW], F32, tag="rinv")
                        nc.scalar.activation(
                            rinv[:], psden[:],
                            mybir.ActivationFunctionType.Exp,
                            scale=-1.0,
                        )
                        norm_pend.append((Qp, rinv))

                    def emit_norm(oT=oT, norm_pend=norm_pend):
                        Qp, rinv_p = norm_pend.pop(0)
                        with nc.allow_low_precision(reason="bf16 out"):
                            nc.vector.tensor_mul(
                                oT[:, Qp * QW : (Qp + 1) * QW],
                                oT[:, Qp * QW : (Qp + 1) * QW],
                                rinv_p[:],
                            )

                    for Q in range(NQ):
                        emit_chunk(h, Q, qT, kT, oT, den_pend, norm_pend,
                                   emit_den, emit_norm)
                    emit_den()
                    emit_norm()
                    if h + 2 < G:
                        prefetch_qk(h + 2)

                for h in range(G):
                    process_head(h)

                assert not fill, f"unemitted fillers: {list(fill)}"
                # final output projection chunk (needs head 7 fully normed)
                for mg in range(4):
                    if mg + 1 < 4:
                        emit_wp_loads(3, mg + 1)
                for m in range(KO):
                    emit_C_m(3, m)

    _split_multi_waits(nc)
    return nc


# --------------------------------------------------------------------------
def _prep_core_inputs(xb, w_attn, w_proj, rope_cos, rope_sin, g, G=8):
    """Host-side shard prep for one core: batch slice xb (T, C), group g."""
    T, C = xb.shape
    KO = C // P
    VN = 512
    NV = (G * P) // VN
    gc = g * G * P  # column offset of this group within one qkv section
    bf = ml_dtypes.bfloat16

    # x^T arranged [p, kc, t]
    xt = np.ascontiguousarray(
        xb.T.reshape(KO, P, T).transpose(1, 0, 2)
    ).astype(bf)

    # q,k columns for this group, RoPE pair-permuted (2i,2i+1) -> (i,64+i),
    # interleaved m: 2h = q head h, 2h+1 = k head h
    perm = np.empty(P, dtype=np.int64)
    perm[:64] = np.arange(0, P, 2)
    perm[64:] = np.arange(1, P, 2)
    wq = w_attn[:, gc : gc + G * P].reshape(C, G, P)[:, :, perm]
    wk = w_attn[:, C + gc : C + gc + G * P].reshape(C, G, P)[:, :, perm]
    wqk_i = np.empty((2 * G, C, P), dtype=np.float32)
    wqk_i[0::2] = wq.transpose(1, 0, 2)
    wqk_i[1::2] = wk.transpose(1, 0, 2)
    # [2G, C, P] -> [2G, P(part), KO, P(cols)]
    wqk = np.ascontiguousarray(
        wqk_i.reshape(2 * G, KO, P, P).transpose(0, 2, 1, 3)
    ).astype(bf)

    wv_cols = w_attn[:, 2 * C + gc : 2 * C + gc + G * P]  # (C, G*128)
    wv = np.ascontiguousarray(
        wv_cols.reshape(KO, P, NV, VN).transpose(2, 1, 0, 3)
    ).astype(bf)

    wp_rows = w_proj[gc : gc + G * P, :]  # (G*128, C)
    wp = np.ascontiguousarray(
        wp_rows.reshape(G, P, KO, P).transpose(2, 1, 0, 3)
    ).astype(bf)

    cT = rope_cos[:T].T  # (64, T)
    sT = rope_sin[:T].T
    cospT = np.ascontiguousarray(np.concatenate([cT, cT], axis=0)).astype(bf)
    sinpT = np.ascontiguousarray(np.concatenate([-sT, sT], axis=0)).astype(bf)
    mask = np.triu(np.ones((P, P), dtype=np.float32)).astype(bf)

    return {
        "xt": xt,
        "wqk": wqk,
        "wv": wv,
        "wp": wp,
        "cosp": cospT,
        "sinp": sinpT,
        "maskt": mask,
        "onesd": np.ones((P, P), dtype=np.float32).astype(bf),
    }


_NC_CACHE = {}
TRACE = False
LAST_RESULTS = None


def kernel(x, w_attn, w_proj, rope_cos, rope_sin):
    from concourse.bass_utils import run_bass_kernel_spmd

    x = np.asarray(x, dtype=np.float32)
    w_attn = np.asarray(w_attn, dtype=np.float32)
    w_proj = np.asarray(w_proj, dtype=np.float32)
    rope_cos = np.asarray(rope_cos, dtype=np.float32)
    rope_sin = np.asarray(rope_sin, dtype=np.float32)

    B, T, C = x.shape
    G = 8  # heads per group (16 heads / 2 groups)

    key = (T, C, G)
    if key not in _NC_CACHE:
        _NC_CACHE[key] = build_attention_core(T=T, C=C, G=G)
    nc = _NC_CACHE[key]

    in_maps = []
    for core in range(8):
        b, g = core // 2, core % 2
        in_maps.append(
            _prep_core_inputs(x[b], w_attn, w_proj, rope_cos, rope_sin, g, G=G)
        )

    res = run_bass_kernel_spmd(nc, in_maps, list(range(8)), trace=TRACE)
    global LAST_RESULTS
    LAST_RESULTS = res

    y = np.empty((B, T, C), dtype=np.float32)
    for b in range(B):
        acc = (
            res.results[2 * b]["outT"].astype(np.float32)
            + res.results[2 * b + 1]["outT"].astype(np.float32)
        )
        y[b] = acc.T
    return y


# revision 73
# speedup vs baseline: 1.0131x; 1.0131x over previous
"""Causal self-attention with RoPE on 8 Trainium2 NeuronCores.

Problem (hardcoded): x (4, 2048, 2048) f32, w_attn (2048, 6144),
w_proj (2048, 2048), rope_cos/rope_sin (2048, 64), 16 heads, hd=128.

Sharding: 8 cores = 4 batches x 2 head-groups (8 heads each).  Each core
computes qkv projection for its heads, RoPE, causal attention, and a
partial output projection (its head-group's rows of w_proj).  The host
sums the two partials per batch (the "all-reduce after c_proj") and
transposes back, since the device kernel works fully transposed.

v3 design notes (PE floor for this decomposition is ~560us/core):
  - bf16 operands everywhere (f32 PSUM accumulation).  Halves SBUF and
    HBM traffic so x (8MB) and v (4MB) stay SBUF-resident for the whole
    kernel; q/k round-trip through DRAM at 0.5MB/head.
  - Attention is locally ACT(exp)-bound: per head the exp+norm chain is
    ~29us of ACT vs ~15us of PE.  Every head therefore gets ACT-free PE
    filler interleaved between its Q-chunks: the v projection (heads
    0-4), the DEFERRED q/k projection of heads 6-7 (heads 2-6), and the
    whole output projection (head 7's tail).  The PE never waits for
    the exp backlog to drain.
  - Softmax denominators: exp tiles are accumulated on Pool/DVE into a
    per-chunk f32r acc tile; ONE ones[128,128] matmul row-sums AND
    broadcasts it, then 1/den = Exp(-Ln(den)) on ACT (Ln/Exp share one
    activation table set; custom-DVE approx reciprocal doesn't compile
    on this walrus build and exact DVE reciprocal costs 3.3us/tile).
  - Normalization of o^T is deferred one Q-chunk so the PE never waits
    on the exp/reciprocal chain.
  - No scalar-engine DMA issues during attention (they would stall the
    saturated ACT stream): phase-B-era DMAs ride the sync/gpsimd queues.
"""

import sys

sys.path.insert(0, "/opt/trn_rl_repo")

import numpy as np
import ml_dtypes

import concourse.bass as bass
import concourse.mybir as mybir
import concourse.tile as tile

F32 = mybir.dt.float32
F32R = mybir.dt.float32r
BF16 = mybir.dt.bfloat16
P = 128


# --------------------------------------------------------------------------
# This container's walrus build rejects any instruction carrying more than
# one sem wait.  Split extras onto NoOps inserted before the instruction on
# the same engine (per-engine program order makes the waits complete first).
def _split_multi_waits(nc):
    n = 0
    for fn in nc.m.functions:
        for bb in fn.blocks:
            out = []
            changed = False
            for inst in bb.instructions:
                si = inst.sync_info
                waits = list(si.on_wait or []) if si is not None else []
                if len(waits) > 1:
                    changed = True
                    n += 1
                    for w in waits[:-1]:
                        nop = mybir.InstNoOp(
                            name=nc.get_next_instruction_name(),
                            engine=inst.engine,
                            ins=[],
                            outs=[],
                            sync_info=mybir.SyncInfo(on_wait=[w], on_update=[]),
                        )
                        try:
                            nc.register_instruction(nop, overwrite=True)
                        except Exception:
                            pass
                        out.append(nop)
                    inst.sync_info = mybir.SyncInfo(
                        on_wait=[waits[-1]], on_update=list(si.on_update or [])
                    )
                out.append(inst)
            if changed:
                bb.instructions = out
    return n


def build_attention_core(T=2048, C=2048, G=8):
    """One core's program.  T tokens, C model dim, G heads in this core's
    group (hd=128 each).  Returns the Bass object."""
    KO = C // P           # 16 contraction tiles over model dim
    NQ = T // 512         # 4 q chunks
    QW = 512
    JPQ = QW // P         # 4 j tiles per q chunk width
    NJ = T // P           # 16 total j tiles
    NTB = NJ              # t blocks for v
    VN = 512              # v column chunk (4 heads)
    NV = (G * P) // VN    # 2 v chunks
    NM_A = 12             # q/k head-tiles computed in phase A (heads 0-5)

    nc = bass.Bass()
    xt = nc.dram_tensor("xt", [P, KO, T], BF16, kind="ExternalInput")
    # m interleaved: 2h = q head h, 2h+1 = k head h (RoPE pair-permuted)
    wqk = nc.dram_tensor("wqk", [2 * G, P, KO, P], BF16, kind="ExternalInput")
    wv = nc.dram_tensor("wv", [NV, P, KO, VN], BF16, kind="ExternalInput")
    wp = nc.dram_tensor("wp", [KO, P, G, P], BF16, kind="ExternalInput")
    # cosd = [cos; cos], sind = [-sin; +sin]  (rope = raw*cosd + swapped*sind)
    cosp = nc.dram_tensor("cosp", [P, T], BF16, kind="ExternalInput")
    sinp = nc.dram_tensor("sinp", [P, T], BF16, kind="ExternalInput")
    maskt = nc.dram_tensor("maskt", [P, P], BF16, kind="ExternalInput")
    onesd = nc.dram_tensor("onesd", [P, P], BF16, kind="ExternalInput")
    outT = nc.dram_tensor("outT", [C, T], BF16, kind="ExternalOutput")

    scale = 1.0 / np.sqrt(128.0)

    with tile.TileContext(nc) as tc:
        with (
            tc.tile_pool(name="dram", bufs=1, space="DRAM") as dram,
            tc.tile_pool(name="const", bufs=1) as cpool,
            tc.tile_pool(name="xres", bufs=1) as x_pool,
            tc.tile_pool(name="vall", bufs=1) as va_pool,
            tc.tile_pool(name="oacc", bufs=G) as oacc_pool,
            tc.tile_pool(name="wv", bufs=1) as wv_pool,
        ):
            qkd = dram.tile([2 * G, P, T], BF16)

            # Start sequence: first head's weights, then x in kc-major
            # slices so the m=0 accumulation can chase the arrivals.
            x_s = x_pool.tile([P, KO, T], BF16, tag="x")
            wqk0 = None  # emitted inside phase A loop; queue order set here
            xsl = [(0, 1), (1, 2), (2, 4), (4, 6), (6, 8), (8, 10),
                   (10, 12), (12, 14), (14, 16)]

            # v resident for phases A+B:
            # v_all[ti, to, hh*128+d] = v[to*128+ti, head hh, d]
            v_all = va_pool.tile([P, NJ, G * P], BF16, tag="vall")

            cos_s = cpool.tile([P, T], BF16)
            sin_s = cpool.tile([P, T], BF16)
            mask_s = cpool.tile([P, P], BF16)
            ones_s = cpool.tile([P, P], BF16)

            # wv0 DMA issues at the very top of the gpsimd queue: the v0
            # blocks open phase B and must not wait behind phase A's swaps
            wv_s = [None, None]
            wv_s[0] = wv_pool.tile([P, KO, VN], BF16, tag="wv", name="wv0")

            # ------------- Phase A: q,k heads 0-5 + RoPE -------------
            with (
                tc.tile_pool(name="wqk", bufs=3) as wqk_pool,
                tc.tile_pool(name="qkraw", bufs=2) as qkraw_pool,
                tc.tile_pool(name="ropesw", bufs=2) as rsw_pool,
                tc.tile_pool(name="roped", bufs=2) as roped_pool,
                tc.tile_pool(name="psA", bufs=2, space="PSUM") as psA,
            ):
                def emit_rope_A(m, pss):
                    # RoPE: rope = raw*[cos;cos] + swap(raw)*[-sin;+sin]
                    raw = qkraw_pool.tile([P, T], BF16, tag="qkraw")
                    nc.scalar.copy(raw[:], pss[:])
                    sw = rsw_pool.tile([P, T], BF16, tag="rsw")
                    nc.gpsimd.dma_start(sw[0:64, :], raw[64:128, :])
                    nc.gpsimd.dma_start(sw[64:128, :], raw[0:64, :])
                    rop = roped_pool.tile([P, T], BF16, tag="roped")
                    with nc.allow_low_precision(reason="bf16 rope"):
                        nc.vector.tensor_mul(rop[:], raw[:], cos_s[:])
                        nc.vector.tensor_mul(sw[:], sw[:], sin_s[:])
                        nc.vector.tensor_add(rop[:], rop[:], sw[:])
                    nc.sync.dma_start(qkd[m], rop[:])

                # m=0,1 run a fused kc loop: one head-tile alone consumes x
                # slower than DMA delivers it, so the PE would stall chasing
                # arrivals; two tiles per kc keep it fed (and fill all 8
                # PSUM banks)
                w01 = []
                for m in range(2):
                    w_s = wqk_pool.tile([P, KO, P], BF16, tag="wqk",
                                        name=f"w{m}")
                    nc.scalar.dma_start(w_s[:, 0:2, :], wqk[m][:, 0:2, :])
                    w01.append(w_s)
                nc.sync.dma_start(x_s[:, 0:1, :], xt[:, 0:1, :])
                nc.scalar.dma_start(w01[0][:, 2:6, :], wqk[0][:, 2:6, :])
                nc.scalar.dma_start(w01[1][:, 2:6, :], wqk[1][:, 2:6, :])
                nc.sync.dma_start(x_s[:, 1:2, :], xt[:, 1:2, :])
                nc.scalar.dma_start(w01[0][:, 6:, :], wqk[0][:, 6:, :])
                nc.scalar.dma_start(w01[1][:, 6:, :], wqk[1][:, 6:, :])
                for a, b in xsl[2:]:
                    nc.sync.dma_start(x_s[:, a:b, :], xt[:, a:b, :])
                # consts/wv0 ride the SYNC ring behind all of x: per-ring
                # FIFO keeps them off the HBM until the kc chase is done
                # (cos/sin land just before m=0's RoPE needs them)
                nc.sync.dma_start(cos_s[:], cosp[:])
                nc.sync.dma_start(sin_s[:], sinp[:])
                nc.sync.dma_start(mask_s[:], maskt[:])
                nc.sync.dma_start(ones_s[:], onesd[:])
                nc.sync.dma_start(wv_s[0][:], wv[0])
                pss01 = [psA.tile([P, T], F32, tag="pqk", name=f"pqk{m}")
                         for m in range(2)]
                for kc in range(KO):
                    for m in range(2):
                        for i in range(NQ):
                            nc.tensor.matmul(
                                pss01[m][:, i * QW : (i + 1) * QW],
                                w01[m][:, kc, :],
                                x_s[:, kc, i * QW : (i + 1) * QW],
                                start=(kc == 0),
                                stop=(kc == KO - 1),
                                skip_group_check=True,
                            )
                emit_rope_A(0, pss01[0])
                emit_rope_A(1, pss01[1])

                for m in range(2, NM_A):
                    w_s = wqk_pool.tile([P, KO, P], BF16, tag="wqk",
                                        name=f"w{m}")
                    nc.scalar.dma_start(w_s[:], wqk[m])
                    pss = psA.tile([P, T], F32, tag="pqk", name=f"pqk{m}")
                    for kc in range(KO):
                        for i in range(NQ):
                            nc.tensor.matmul(
                                pss[:, i * QW : (i + 1) * QW],
                                w_s[:, kc, :],
                                x_s[:, kc, i * QW : (i + 1) * QW],
                                start=(kc == 0),
                                stop=(kc == KO - 1),
                                skip_group_check=True,
                            )
                    emit_rope_A(m, pss)

            # ------- Phase B/C: v, attention, deferred q/k, out proj -------
            with (
                tc.tile_pool(name="qh", bufs=2) as q_pool,
                tc.tile_pool(name="kh", bufs=2) as k_pool,
                tc.tile_pool(name="pt", bufs=6) as pt_pool,
                tc.tile_pool(name="acc", bufs=2) as acc_pool,
                tc.tile_pool(name="accb", bufs=1) as accb_pool,
                tc.tile_pool(name="rinv", bufs=2) as rinv_pool,
                tc.tile_pool(name="dq_w", bufs=2) as dqw_pool,
                tc.tile_pool(name="dq_t", bufs=1) as dqt_pool,
                tc.tile_pool(name="wp", bufs=4) as wp_pool,
                tc.tile_pool(name="csb", bufs=2) as csb_pool,
                tc.tile_pool(name="psS", bufs=4, space="PSUM") as psS,
                tc.tile_pool(name="psO", bufs=2, space="PSUM") as psO,
                tc.tile_pool(name="psM", bufs=2, space="PSUM") as psM,
            ):
                def emit_v_block(n2, tb):
                    psv = psM.tile([P, VN], F32, tag="psm")
                    for kc in range(KO):
                        nc.tensor.matmul(
                            psv[:],
                            x_s[:, kc, tb * P : (tb + 1) * P],
                            wv_s[n2][:, kc, :],
                            start=(kc == 0),
                            stop=(kc == KO - 1),
                        )
                    # Pool can't read PSUM on trn2; DVE takes this copy
                    with nc.allow_low_precision(reason="v is bf16"):
                        nc.vector.tensor_copy(
                            v_all[:, tb, n2 * VN : (n2 + 1) * VN], psv[:]
                        )

                def emit_wv1_load():
                    wv_s[1] = wv_pool.tile([P, KO, VN], BF16, tag="wv",
                                           name="wv1")
                    nc.gpsimd.dma_start(wv_s[1][:], wv[1])

                # deferred q/k head-tile, in 512-wide chunks ([128,512]
                # PSUM ring; phase A's 4-bank tiles don't fit here).  m=15
                # (k of head 7) writes straight into head 7's SBUF k tile:
                # a DRAM round-trip would race the ring-buffer prefetch.
                def emit_dqk_load(m):
                    dw = dqw_pool.tile([P, KO, P], BF16, tag="dw",
                                       name=f"dw{m}")
                    nc.sync.dma_start(dw[:], wqk[m])
                    emit_dqk_chunk.w[m] = dw

                def emit_dqk_chunk(m, c):
                    sl = slice(c * QW, (c + 1) * QW)
                    if m not in emit_dqk_chunk.w:
                        emit_dqk_load(m)
                    dw = emit_dqk_chunk.w[m]
                    psq = psM.tile([P, QW], F32, tag="psm")
                    for kc in range(KO):
                        nc.tensor.matmul(
                            psq[:], dw[:, kc, :], x_s[:, kc, sl],
                            start=(kc == 0), stop=(kc == KO - 1),
                            skip_group_check=True,
                        )
                    raw = dqt_pool.tile([P, QW], BF16, tag="draw")
                    with nc.allow_low_precision(reason="bf16 rope"):
                        nc.vector.tensor_copy(raw[:], psq[:])
                    sw = dqt_pool.tile([P, QW], BF16, tag="dsw")
                    nc.gpsimd.dma_start(sw[0:64, :], raw[64:128, :])
                    nc.gpsimd.dma_start(sw[64:128, :], raw[0:64, :])
                    if m == 15:
                        if 7 not in kts:
                            kts[7] = k_pool.tile([P, T], BF16, tag="k",
                                                 name="k7")
                        rop_out = kts[7][:, sl]
                    else:
                        rop_out = None
                    rop = dqt_pool.tile([P, QW], BF16, tag="drop")
                    with nc.allow_low_precision(reason="bf16 rope"):
                        nc.vector.tensor_mul(rop[:], raw[:], cos_s[:, sl])
                        nc.vector.tensor_mul(sw[:], sw[:], sin_s[:, sl])
                        if rop_out is not None:
                            nc.vector.tensor_add(rop_out, rop[:], sw[:])
                        else:
                            nc.vector.tensor_add(rop[:], rop[:], sw[:])
                            nc.sync.dma_start(qkd[m][:, sl], rop[:])

                emit_dqk_chunk.w = {}

                # phase C as fine-grained per-m pieces so they can weave
                # into head 7's J-loops; wp streams through a 4-slot ring,
                # loaded one quad ahead of its consumers
                wp_tiles = {}

                def emit_wp_loads(t, mg):
                    for m in range(mg * 4, mg * 4 + 4):
                        wpt = wp_pool.tile([P, G, P], BF16, tag="wp",
                                           name=f"wp{t}_{m}")
                        nc.sync.dma_start(wpt[:], wp[m])
                        wp_tiles[(t, m)] = wpt

                def emit_C_m(t, m):
                    psc = psM.tile([P, QW], F32, tag="psm")
                    wpt = wp_tiles.pop((t, m))
                    for hh in range(G):
                        nc.tensor.matmul(
                            psc[:],
                            wpt[:, hh, :],
                            oTs[hh][:, t * QW : (t + 1) * QW],
                            start=(hh == 0),
                            stop=(hh == G - 1),
                        )
                    csb = csb_pool.tile([P, QW], BF16, tag="csb")
                    # alternate engines: DVE alone backs up during the
                    # C-era, but ACT can't take them all either (C pieces
                    # weave into head 7's exp-saturated chunks)
                    if m % 2 == 0:
                        with nc.allow_low_precision(reason="bf16 out"):
                            nc.vector.tensor_copy(csb[:], psc[:])
                    else:
                        nc.scalar.copy(csb[:], psc[:])
                    nc.sync.dma_start(
                        outT[m * P : (m + 1) * P, t * QW : (t + 1) * QW],
                        csb[:],
                    )

                def add_C_t(h, q, t):
                    # per t-chunk: 16 m pieces with the NEXT quad's wp load
                    # emitted before each quad's pieces (ring stays 1 ahead)
                    for mg in range(4):
                        if mg + 1 < 4:
                            add(h, q, emit_wp_loads, t, mg + 1)
                        elif t + 1 < NQ:
                            add(h, q, emit_wp_loads, t + 1, 0)
                        for m in range(mg * 4, mg * 4 + 4):
                            add(h, q, emit_C_m, t, m)

                # ---- PE filler schedule: slot (h, Q) -> list of thunks ----
                # v0 = v heads 0-3 (16 blocks), v1 = v heads 4-7 (16),
                # dqk m=12..15 = q/k heads 6-7, C = output projection.
                # Constraints: v0 tb in [4Q,4Q+3] before (0..3, Q); v1 tb
                # likewise before (4, Q); dqk m before head m//2; C t-chunk
                # after head 7's Q=t norm (deferred: emitted during Q=t+1).
                fill = {}

                def add(h, q, fn, *a):
                    fill.setdefault((h, q), []).append((fn, a))

                for tb in range(4, 8):
                    add(0, 0, emit_v_block, 0, tb)
                for tb in range(8, 12):
                    add(0, 1, emit_v_block, 0, tb)
                for tb in range(12, 16):
                    add(0, 2, emit_v_block, 0, tb)
                add(0, 3, emit_wv1_load)
                for tb in range(0, 3):
                    add(1, tb + 1, emit_v_block, 1, tb)
                for tb in range(3, 7):
                    add(2, tb - 3, emit_v_block, 1, tb)
                add(1, 3, emit_dqk_load, 12)
                add(2, 3, emit_dqk_load, 13)
                add(4, 3, emit_dqk_load, 14)
                add(5, 3, emit_dqk_load, 15)
                for i, c in enumerate(range(4)):     # m=12 in head 2/3
                    add(2 if i < 2 else 3, i % 2 * 2, emit_dqk_chunk, 12, c)
                for tb in range(7, 11):
                    add(3, tb - 7, emit_v_block, 1, tb)
                for c in range(4):                   # m=13 in head 3/4
                    add(3 if c < 2 else 4, c % 2, emit_dqk_chunk, 13, c)
                # (4,Q) consumes v1 tb<=4Q+3, so tb12-15 must precede (4,3)
                for tb, q in [(11, 0), (12, 0), (13, 1), (14, 1), (15, 2)]:
                    add(4, q, emit_v_block, 1, tb)
                for c in range(4):                   # m=14 in head 5
                    add(5, c, emit_dqk_chunk, 14, c)
                for c in range(4):                   # m=15 in head 6
                    add(6, c, emit_dqk_chunk, 15, c)
                add(6, 3, emit_wp_loads, 0, 0)
                add_C_t(7, 1, 0)
                add_C_t(7, 2, 1)
                add_C_t(7, 3, 2)

                # v0 tb0-3 must precede head 0 entirely
                for tb in range(4):
                    emit_v_block(0, tb)

                # q/k prefetch for heads 0,1 (heads 2+ prefetched inside the
                # loop).  Head 7's k never touches DRAM (m=15 writes its
                # SBUF tile directly), and its q prefetch is emitted after
                # m=14's qkd writes, so program order keeps RAW intact.
                qts, kts = {}, {}

                def prefetch_q(hp):
                    qts[hp] = q_pool.tile([P, T], BF16, tag="q",
                                          name=f"q{hp}")
                    nc.sync.dma_start(qts[hp][:], qkd[2 * hp])

                def prefetch_k(hp):
                    kts[hp] = k_pool.tile([P, T], BF16, tag="k",
                                          name=f"k{hp}")
                    nc.sync.dma_start(kts[hp][:], qkd[2 * hp + 1])

                def prefetch_qk(hp):
                    prefetch_q(hp)
                    if hp != 7:
                        prefetch_k(hp)

                prefetch_qk(0)
                prefetch_qk(1)

                def emit_attn_J(h, Q, J, qT, kT, acc_a, acc_b):
                    k_d = J - JPQ * Q  # diag idx if >= 0
                    co = max(k_d, 0) * P
                    pss = psS.tile([P, QW], F32, tag="pss")
                    nc.tensor.matmul(
                        pss[:, co:],
                        kT[:, J * P : (J + 1) * P],
                        qT[:, Q * QW + co : (Q + 1) * QW],
                        start=True,
                        stop=True,
                        skip_group_check=True,
                    )
                    pT = pt_pool.tile([P, QW], BF16, tag="pt")
                    nc.scalar.activation(
                        pT[:, co:], pss[:, co:],
                        mybir.ActivationFunctionType.Exp,
                        scale=scale,
                    )
                    if k_d >= 0:
                        with nc.allow_low_precision(reason="bf16 attn"):
                            nc.vector.tensor_mul(
                                pT[:, co : co + P],
                                pT[:, co : co + P],
                                mask_s[:],
                            )
                    # den chain split across DVE (even J) and Pool (odd J):
                    # either engine alone can't keep up with the exp cadence
                    # and a lagging chain holds PSUM ring slots, stalling PE
                    with nc.allow_low_precision(reason="f32 bits"):
                        if J == 0:
                            nc.vector.tensor_copy(acc_a[:], pT[:])
                        elif J == 1:
                            nc.gpsimd.tensor_copy(
                                acc_b[:, co:], pT[:, co:]
                            )
                        elif J % 2 == 0:
                            nc.vector.tensor_add(
                                acc_a[:, co:], acc_a[:, co:], pT[:, co:]
                            )
                        else:
                            nc.gpsimd.tensor_add(
                                acc_b[:, co:], acc_b[:, co:], pT[:, co:]
                            )
                    return co, pT

                def emit_chunk(h, Q, qT, kT, oT, den_pend, norm_pend,
                               emit_den, emit_norm):
                    jmax = JPQ * (Q + 1) - 1  # inclusive
                    # previous chunk's den reduction + norm run FIRST: the
                    # combine drained behind the previous chunk's work, and
                    # with the norm done up front, this slot's fillers
                    # (incl. phase C pieces at head 7) are legal to weave
                    # into the J-loop below.
                    emit_den()
                    if norm_pend:
                        emit_norm()
                    pso = psO.tile([P, QW], F32, tag="pso")
                    acc_a = acc_pool.tile([P, QW], BF16, tag="acca")
                    acc_b = accb_pool.tile([P, QW], BF16, tag="accb")
                    # PV(J) consumes exp(S(J)) from ACT; emit it LOOK
                    # S-matmuls later so PE never stalls on ACT.  The exp
                    # cadence (~535ns) exceeds the PE's per-J work (~426ns),
                    # so ACT-independent filler is woven in every few J's
                    # to keep the PE fed while ACT catches up.
                    LOOK = 3
                    fillers = fill.pop((h, Q), [])
                    pend = []
                    for J in range(jmax + 1 + LOOK):
                        if J <= jmax:
                            pend.append(
                                (J,)
                                + emit_attn_J(h, Q, J, qT, kT, acc_a, acc_b)
                            )
                        if J >= LOOK:
                            Jp, cop, pTp = pend.pop(0)
                            nc.tensor.matmul(
                                pso[:, cop:],
                                v_all[:, Jp, h * P : (h + 1) * P],
                                pTp[:, cop:],
                                start=(Jp == 0),
                                stop=(Jp == jmax),
                                skip_group_check=True,
                            )
                        if J % 4 == 3 and fillers:
                            fn, a = fillers.pop(0)
                            fn(*a)
                    # raw o^T out of PSUM (DVE: ACT is exp-saturated here)
                    with nc.allow_low_precision(reason="bf16 out"):
                        nc.vector.tensor_copy(
                            oT[:, Q * QW : (Q + 1) * QW], pso[:]
                        )
                    # den combine runs on DVE NOW (right behind this chunk's
                    # own adds, so it drains ASAP), but the dependent psden
                    # matmul is deferred one chunk (emit_den above) so the
                    # in-order PE never waits on the chain.
                    # acc_b is initialized at J=1, whose co is 128 for Q=0
                    # (columns below have no odd-J contributions)
                    cb = P if Q == 0 else 0
                    with nc.allow_low_precision(reason="f32 bits"):
                        nc.vector.tensor_add(
                            acc_a[:, cb:], acc_a[:, cb:], acc_b[:, cb:]
                        )
                    den_pend.append((Q, acc_a))
                    # remaining PE filler for this slot
                    for fn, a in fillers:
                        fn(*a)

                oTs = []

                def process_head(h):
                    qT = qts.pop(h)
                    kT = kts.pop(h)
                    oT = oacc_pool.tile([P, T], BF16, tag="oacc",
                                        name=f"oT{h}")
                    oTs.append(oT)

                    # deferred den-reduction / normalization state
                    den_pend = []
                    norm_pend = []

                    def emit_den(den_pend=den_pend, norm_pend=norm_pend):
                        if not den_pend:
                            return
                        Qp, acc_a = den_pend.pop(0)
                        psden = psM.tile([P, QW], F32, tag="psm")
                        nc.tensor.matmul(
                            psden[:], ones_s[:], acc_a[:],
                            start=True, stop=True, skip_group_check=True,
                        )
                        nc.scalar.activation(
                            psden[:], psden[:],
                            mybir.ActivationFunctionType.Ln,
                        )
                        rinv = rinv_pool.tile([P, QW], F32, tag="rinv")
                        nc.scalar.activation(
                            rinv[:], psden[:],
                            mybir.ActivationFunctionType.Exp,
                            scale=-1.0,
                        )
                        norm_pend.append((Qp, rinv))

                    def emit_norm(oT=oT, norm_pend=norm_pend):
                        Qp, rinv_p = norm_pend.pop(0)
                        with nc.allow_low_precision(reason="bf16 out"):
                            nc.vector.tensor_mul(
                                oT[:, Qp * QW : (Qp + 1) * QW],
                                oT[:, Qp * QW : (Qp + 1) * QW],
                                rinv_p[:],
                            )

                    for Q in range(NQ):
                        emit_chunk(h, Q, qT, kT, oT, den_pend, norm_pend,
                                   emit_den, emit_norm)
                    emit_den()
                    emit_norm()
                    if h + 2 < G:
                        prefetch_qk(h + 2)

                for h in range(G):
                    process_head(h)

                assert not fill, f"unemitted fillers: {list(fill)}"
                # final output projection chunk (needs head 7 fully normed)
                for mg in range(4):
                    if mg + 1 < 4:
                        emit_wp_loads(3, mg + 1)
                for m in range(KO):
                    emit_C_m(3, m)

    _split_multi_waits(nc)
    return nc


# --------------------------------------------------------------------------
def _prep_core_inputs(xb, w_attn, w_proj, rope_cos, rope_sin, g, G=8):
    """Host-side shard prep for one core: batch slice xb (T, C), group g."""
    T, C = xb.shape
    KO = C // P
    VN = 512
    NV = (G * P) // VN
    gc = g * G * P  # column offset of this group within one qkv section
    bf = ml_dtypes.bfloat16

    # x^T arranged [p, kc, t]
    xt = np.ascontiguousarray(
        xb.T.reshape(KO, P, T).transpose(1, 0, 2)
    ).astype(bf)

    # q,k columns for this group, RoPE pair-permuted (2i,2i+1) -> (i,64+i),
    # interleaved m: 2h = q head h, 2h+1 = k head h
    perm = np.empty(P, dtype=np.int64)
    perm[:64] = np.arange(0, P, 2)
    perm[64:] = np.arange(1, P, 2)
    wq = w_attn[:, gc : gc + G * P].reshape(C, G, P)[:, :, perm]
    wk = w_attn[:, C + gc : C + gc + G * P].reshape(C, G, P)[:, :, perm]
    wqk_i = np.empty((2 * G, C, P), dtype=np.float32)
    wqk_i[0::2] = wq.transpose(1, 0, 2)
    wqk_i[1::2] = wk.transpose(1, 0, 2)
    # [2G, C, P] -> [2G, P(part), KO, P(cols)]
    wqk = np.ascontiguousarray(
        wqk_i.reshape(2 * G, KO, P, P).transpose(0, 2, 1, 3)
    ).astype(bf)

    wv_cols = w_attn[:, 2 * C + gc : 2 * C + gc + G * P]  # (C, G*128)
    wv = np.ascontiguousarray(
        wv_cols.reshape(KO, P, NV, VN).transpose(2, 1, 0, 3)
    ).astype(bf)

    wp_rows = w_proj[gc : gc + G * P, :]  # (G*128, C)
    wp = np.ascontiguousarray(
        wp_rows.reshape(G, P, KO, P).transpose(2, 1, 0, 3)
    ).astype(bf)

    cT = rope_cos[:T].T  # (64, T)
    sT = rope_sin[:T].T
    cospT = np.ascontiguousarray(np.concatenate([cT, cT], axis=0)).astype(bf)
    sinpT = np.ascontiguousarray(np.concatenate([-sT, sT], axis=0)).astype(bf)
    mask = np.triu(np.ones((P, P), dtype=np.float32)).astype(bf)

    return {
        "xt": xt,
        "wqk": wqk,
        "wv": wv,
        "wp": wp,
        "cosp": cospT,
        "sinp": sinpT,
        "maskt": mask,
        "onesd": np.ones((P, P), dtype=np.float32).astype(bf),
    }


_NC_CACHE = {}
TRACE = False
LAST_RESULTS = None


def kernel(x, w_attn, w_proj, rope_cos, rope_sin):
    from concourse.bass_utils import run_bass_kernel_spmd

    x = np.asarray(x, dtype=np.float32)
    w_attn = np.asarray(w_attn, dtype=np.float32)
    w_proj = np.asarray(w_proj, dtype=np.float32)
    rope_cos = np.asarray(rope_cos, dtype=np.float32)
    rope_sin = np.asarray(rope_sin, dtype=np.float32)

    B, T, C = x.shape
    G = 8  # heads per group (16 heads / 2 groups)

    key = (T, C, G)
    if key not in _NC_CACHE:
        _NC_CACHE[key] = build_attention_core(T=T, C=C, G=G)
    nc = _NC_CACHE[key]

    in_maps = []
    for core in range(8):
        b, g = core // 2, core % 2
        in_maps.append(
            _prep_core_inputs(x[b], w_attn, w_proj, rope_cos, rope_sin, g, G=G)
        )

    res = run_bass_kernel_spmd(nc, in_maps, list(range(8)), trace=TRACE)
    global LAST_RESULTS
    LAST_RESULTS = res

    y = np.empty((B, T, C), dtype=np.float32)
    for b in range(B):
        acc = (
            res.results[2 * b]["outT"].astype(np.float32)
            + res.results[2 * b + 1]["outT"].astype(np.float32)
        )
        y[b] = acc.T
    return y


# revision 74
# speedup vs baseline: 1.0188x; 1.0056x over previous
"""Causal self-attention with RoPE on 8 Trainium2 NeuronCores.

Problem (hardcoded): x (4, 2048, 2048) f32, w_attn (2048, 6144),
w_proj (2048, 2048), rope_cos/rope_sin (2048, 64), 16 heads, hd=128.

Sharding: 8 cores = 4 batches x 2 head-groups (8 heads each).  Each core
computes qkv projection for its heads, RoPE, causal attention, and a
partial output projection (its head-group's rows of w_proj).  The host
sums the two partials per batch (the "all-reduce after c_proj") and
transposes back, since the device kernel works fully transposed.

v3 design notes (PE floor for this decomposition is ~560us/core):
  - bf16 operands everywhere (f32 PSUM accumulation).  Halves SBUF and
    HBM traffic so x (8MB) and v (4MB) stay SBUF-resident for the whole
    kernel; q/k round-trip through DRAM at 0.5MB/head.
  - Attention is locally ACT(exp)-bound: per head the exp+norm chain is
    ~29us of ACT vs ~15us of PE.  Every head therefore gets ACT-free PE
    filler interleaved between its Q-chunks: the v projection (heads
    0-4), the DEFERRED q/k projection of heads 6-7 (heads 2-6), and the
    whole output projection (head 7's tail).  The PE never waits for
    the exp backlog to drain.
  - Softmax denominators: exp tiles are accumulated on Pool/DVE into a
    per-chunk f32r acc tile; ONE ones[128,128] matmul row-sums AND
    broadcasts it, then 1/den = Exp(-Ln(den)) on ACT (Ln/Exp share one
    activation table set; custom-DVE approx reciprocal doesn't compile
    on this walrus build and exact DVE reciprocal costs 3.3us/tile).
  - Normalization of o^T is deferred one Q-chunk so the PE never waits
    on the exp/reciprocal chain.
  - No scalar-engine DMA issues during attention (they would stall the
    saturated ACT stream): phase-B-era DMAs ride the sync/gpsimd queues.
"""

import sys

sys.path.insert(0, "/opt/trn_rl_repo")

import numpy as np
import ml_dtypes

import concourse.bass as bass
import concourse.mybir as mybir
import concourse.tile as tile

F32 = mybir.dt.float32
F32R = mybir.dt.float32r
BF16 = mybir.dt.bfloat16
P = 128


# --------------------------------------------------------------------------
# This container's walrus build rejects any instruction carrying more than
# one sem wait.  Split extras onto NoOps inserted before the instruction on
# the same engine (per-engine program order makes the waits complete first).
def _split_multi_waits(nc):
    n = 0
    for fn in nc.m.functions:
        for bb in fn.blocks:
            out = []
            changed = False
            for inst in bb.instructions:
                si = inst.sync_info
                waits = list(si.on_wait or []) if si is not None else []
                if len(waits) > 1:
                    changed = True
                    n += 1
                    for w in waits[:-1]:
                        nop = mybir.InstNoOp(
                            name=nc.get_next_instruction_name(),
                            engine=inst.engine,
                            ins=[],
                            outs=[],
                            sync_info=mybir.SyncInfo(on_wait=[w], on_update=[]),
                        )
                        try:
                            nc.register_instruction(nop, overwrite=True)
                        except Exception:
                            pass
                        out.append(nop)
                    inst.sync_info = mybir.SyncInfo(
                        on_wait=[waits[-1]], on_update=list(si.on_update or [])
                    )
                out.append(inst)
            if changed:
                bb.instructions = out
    return n


def build_attention_core(T=2048, C=2048, G=8):
    """One core's program.  T tokens, C model dim, G heads in this core's
    group (hd=128 each).  Returns the Bass object."""
    KO = C // P           # 16 contraction tiles over model dim
    NQ = T // 512         # 4 q chunks
    QW = 512
    JPQ = QW // P         # 4 j tiles per q chunk width
    NJ = T // P           # 16 total j tiles
    NTB = NJ              # t blocks for v
    VN = 512              # v column chunk (4 heads)
    NV = (G * P) // VN    # 2 v chunks
    NM_A = 12             # q/k head-tiles computed in phase A (heads 0-5)

    nc = bass.Bass()
    xt = nc.dram_tensor("xt", [P, KO, T], BF16, kind="ExternalInput")
    # m interleaved: 2h = q head h, 2h+1 = k head h (RoPE pair-permuted)
    wqk = nc.dram_tensor("wqk", [2 * G, P, KO, P], BF16, kind="ExternalInput")
    wv = nc.dram_tensor("wv", [NV, P, KO, VN], BF16, kind="ExternalInput")
    wp = nc.dram_tensor("wp", [KO, P, G, P], BF16, kind="ExternalInput")
    # cosd = [cos; cos], sind = [-sin; +sin]  (rope = raw*cosd + swapped*sind)
    cosp = nc.dram_tensor("cosp", [P, T], BF16, kind="ExternalInput")
    sinp = nc.dram_tensor("sinp", [P, T], BF16, kind="ExternalInput")
    maskt = nc.dram_tensor("maskt", [P, P], BF16, kind="ExternalInput")
    onesd = nc.dram_tensor("onesd", [P, P], BF16, kind="ExternalInput")
    outT = nc.dram_tensor("outT", [C, T], BF16, kind="ExternalOutput")

    scale = 1.0 / np.sqrt(128.0)

    with tile.TileContext(nc) as tc:
        with (
            tc.tile_pool(name="dram", bufs=1, space="DRAM") as dram,
            tc.tile_pool(name="const", bufs=1) as cpool,
            tc.tile_pool(name="xres", bufs=1) as x_pool,
            tc.tile_pool(name="vall", bufs=1) as va_pool,
            tc.tile_pool(name="oacc", bufs=G) as oacc_pool,
            tc.tile_pool(name="wv", bufs=1) as wv_pool,
        ):
            qkd = dram.tile([2 * G, P, T], BF16)

            # Start sequence: first head's weights, then x in kc-major
            # slices so the m=0 accumulation can chase the arrivals.
            x_s = x_pool.tile([P, KO, T], BF16, tag="x")
            wqk0 = None  # emitted inside phase A loop; queue order set here
            xsl = [(0, 1), (1, 2), (2, 4), (4, 6), (6, 8), (8, 10),
                   (10, 12), (12, 14), (14, 16)]

            # v resident for phases A+B:
            # v_all[ti, to, hh*128+d] = v[to*128+ti, head hh, d]
            v_all = va_pool.tile([P, NJ, G * P], BF16, tag="vall")

            cos_s = cpool.tile([P, T], BF16)
            sin_s = cpool.tile([P, T], BF16)
            mask_s = cpool.tile([P, P], BF16)
            ones_s = cpool.tile([P, P], BF16)

            # wv0 DMA issues at the very top of the gpsimd queue: the v0
            # blocks open phase B and must not wait behind phase A's swaps
            wv_s = [None, None]
            wv_s[0] = wv_pool.tile([P, KO, VN], BF16, tag="wv", name="wv0")

            # ------------- Phase A: q,k heads 0-5 + RoPE -------------
            with (
                tc.tile_pool(name="wqk", bufs=3) as wqk_pool,
                tc.tile_pool(name="qkraw", bufs=2) as qkraw_pool,
                tc.tile_pool(name="ropesw", bufs=2) as rsw_pool,
                tc.tile_pool(name="roped", bufs=2) as roped_pool,
                tc.tile_pool(name="psA", bufs=2, space="PSUM") as psA,
            ):
                def emit_rope_A(m, pss):
                    # RoPE: rope = raw*[cos;cos] + swap(raw)*[-sin;+sin]
                    raw = qkraw_pool.tile([P, T], BF16, tag="qkraw")
                    nc.scalar.copy(raw[:], pss[:])
                    sw = rsw_pool.tile([P, T], BF16, tag="rsw")
                    nc.gpsimd.dma_start(sw[0:64, :], raw[64:128, :])
                    nc.gpsimd.dma_start(sw[64:128, :], raw[0:64, :])
                    rop = roped_pool.tile([P, T], BF16, tag="roped")
                    with nc.allow_low_precision(reason="bf16 rope"):
                        nc.vector.tensor_mul(rop[:], raw[:], cos_s[:])
                        nc.vector.tensor_mul(sw[:], sw[:], sin_s[:])
                        nc.vector.tensor_add(rop[:], rop[:], sw[:])
                    nc.sync.dma_start(qkd[m], rop[:])

                # m=0,1 run a fused kc loop: one head-tile alone consumes x
                # slower than DMA delivers it, so the PE would stall chasing
                # arrivals; two tiles per kc keep it fed (and fill all 8
                # PSUM banks)
                w01 = []
                for m in range(2):
                    w_s = wqk_pool.tile([P, KO, P], BF16, tag="wqk",
                                        name=f"w{m}")
                    nc.scalar.dma_start(w_s[:, 0:2, :], wqk[m][:, 0:2, :])
                    w01.append(w_s)
                nc.sync.dma_start(x_s[:, 0:1, :], xt[:, 0:1, :])
                nc.scalar.dma_start(w01[0][:, 2:6, :], wqk[0][:, 2:6, :])
                nc.scalar.dma_start(w01[1][:, 2:6, :], wqk[1][:, 2:6, :])
                nc.sync.dma_start(x_s[:, 1:2, :], xt[:, 1:2, :])
                nc.scalar.dma_start(w01[0][:, 6:, :], wqk[0][:, 6:, :])
                nc.scalar.dma_start(w01[1][:, 6:, :], wqk[1][:, 6:, :])
                for a, b in xsl[2:]:
                    nc.sync.dma_start(x_s[:, a:b, :], xt[:, a:b, :])
                # consts/wv0 ride the SYNC ring behind all of x: per-ring
                # FIFO keeps them off the HBM until the kc chase is done
                # (cos/sin land just before m=0's RoPE needs them)
                nc.sync.dma_start(cos_s[:], cosp[:])
                nc.sync.dma_start(sin_s[:], sinp[:])
                nc.sync.dma_start(mask_s[:], maskt[:])
                nc.sync.dma_start(ones_s[:], onesd[:])
                nc.sync.dma_start(wv_s[0][:], wv[0])
                pss01 = [psA.tile([P, T], F32, tag="pqk", name=f"pqk{m}")
                         for m in range(2)]
                for kc in range(KO):
                    for m in range(2):
                        for i in range(NQ):
                            nc.tensor.matmul(
                                pss01[m][:, i * QW : (i + 1) * QW],
                                w01[m][:, kc, :],
                                x_s[:, kc, i * QW : (i + 1) * QW],
                                start=(kc == 0),
                                stop=(kc == KO - 1),
                                skip_group_check=True,
                            )
                emit_rope_A(0, pss01[0])
                emit_rope_A(1, pss01[1])

                for m in range(2, NM_A):
                    w_s = wqk_pool.tile([P, KO, P], BF16, tag="wqk",
                                        name=f"w{m}")
                    nc.scalar.dma_start(w_s[:], wqk[m])
                    pss = psA.tile([P, T], F32, tag="pqk", name=f"pqk{m}")
                    for kc in range(KO):
                        for i in range(NQ):
                            nc.tensor.matmul(
                                pss[:, i * QW : (i + 1) * QW],
                                w_s[:, kc, :],
                                x_s[:, kc, i * QW : (i + 1) * QW],
                                start=(kc == 0),
                                stop=(kc == KO - 1),
                                skip_group_check=True,
                            )
                    emit_rope_A(m, pss)

            # ------- Phase B/C: v, attention, deferred q/k, out proj -------
            with (
                tc.tile_pool(name="qh", bufs=2) as q_pool,
                tc.tile_pool(name="kh", bufs=2) as k_pool,
                tc.tile_pool(name="pt", bufs=6) as pt_pool,
                tc.tile_pool(name="acc", bufs=2) as acc_pool,
                tc.tile_pool(name="accb", bufs=1) as accb_pool,
                tc.tile_pool(name="rinv", bufs=2) as rinv_pool,
                tc.tile_pool(name="dq_w", bufs=2) as dqw_pool,
                tc.tile_pool(name="dq_t", bufs=1) as dqt_pool,
                tc.tile_pool(name="wp", bufs=4) as wp_pool,
                tc.tile_pool(name="csb", bufs=2) as csb_pool,
                tc.tile_pool(name="psS", bufs=4, space="PSUM") as psS,
                tc.tile_pool(name="psO", bufs=2, space="PSUM") as psO,
                tc.tile_pool(name="psM", bufs=2, space="PSUM") as psM,
            ):
                def emit_v_block(n2, tb):
                    psv = psM.tile([P, VN], F32, tag="psm")
                    for kc in range(KO):
                        nc.tensor.matmul(
                            psv[:],
                            x_s[:, kc, tb * P : (tb + 1) * P],
                            wv_s[n2][:, kc, :],
                            start=(kc == 0),
                            stop=(kc == KO - 1),
                        )
                    # Pool can't read PSUM on trn2; DVE takes this copy
                    with nc.allow_low_precision(reason="v is bf16"):
                        nc.vector.tensor_copy(
                            v_all[:, tb, n2 * VN : (n2 + 1) * VN], psv[:]
                        )

                def emit_wv1_load():
                    wv_s[1] = wv_pool.tile([P, KO, VN], BF16, tag="wv",
                                           name="wv1")
                    nc.gpsimd.dma_start(wv_s[1][:], wv[1])

                # deferred q/k head-tile, in 512-wide chunks ([128,512]
                # PSUM ring; phase A's 4-bank tiles don't fit here).  m=15
                # (k of head 7) writes straight into head 7's SBUF k tile:
                # a DRAM round-trip would race the ring-buffer prefetch.
                def emit_dqk_load(m):
                    dw = dqw_pool.tile([P, KO, P], BF16, tag="dw",
                                       name=f"dw{m}")
                    nc.sync.dma_start(dw[:], wqk[m])
                    emit_dqk_chunk.w[m] = dw

                def emit_dqk_chunk(m, c):
                    sl = slice(c * QW, (c + 1) * QW)
                    if m not in emit_dqk_chunk.w:
                        emit_dqk_load(m)
                    dw = emit_dqk_chunk.w[m]
                    psq = psM.tile([P, QW], F32, tag="psm")
                    for kc in range(KO):
                        nc.tensor.matmul(
                            psq[:], dw[:, kc, :], x_s[:, kc, sl],
                            start=(kc == 0), stop=(kc == KO - 1),
                            skip_group_check=True,
                        )
                    raw = dqt_pool.tile([P, QW], BF16, tag="draw")
                    with nc.allow_low_precision(reason="bf16 rope"):
                        nc.vector.tensor_copy(raw[:], psq[:])
                    sw = dqt_pool.tile([P, QW], BF16, tag="dsw")
                    nc.gpsimd.dma_start(sw[0:64, :], raw[64:128, :])
                    nc.gpsimd.dma_start(sw[64:128, :], raw[0:64, :])
                    if m == 15:
                        if 7 not in kts:
                            kts[7] = k_pool.tile([P, T], BF16, tag="k",
                                                 name="k7")
                        rop_out = kts[7][:, sl]
                    else:
                        rop_out = None
                    rop = dqt_pool.tile([P, QW], BF16, tag="drop")
                    with nc.allow_low_precision(reason="bf16 rope"):
                        nc.vector.tensor_mul(rop[:], raw[:], cos_s[:, sl])
                        nc.vector.tensor_mul(sw[:], sw[:], sin_s[:, sl])
                        if rop_out is not None:
                            nc.vector.tensor_add(rop_out, rop[:], sw[:])
                        else:
                            nc.vector.tensor_add(rop[:], rop[:], sw[:])
                            nc.sync.dma_start(qkd[m][:, sl], rop[:])

                emit_dqk_chunk.w = {}

                # phase C as fine-grained per-m pieces so they can weave
                # into head 7's J-loops; wp streams through a 4-slot ring,
                # loaded one quad ahead of its consumers
                wp_tiles = {}

                def emit_wp_loads(t, mg):
                    for m in range(mg * 4, mg * 4 + 4):
                        wpt = wp_pool.tile([P, G, P], BF16, tag="wp",
                                           name=f"wp{t}_{m}")
                        nc.sync.dma_start(wpt[:], wp[m])
                        wp_tiles[(t, m)] = wpt

                def emit_C_m(t, m):
                    psc = psM.tile([P, QW], F32, tag="psm")
                    wpt = wp_tiles.pop((t, m))
                    for hh in range(G):
                        nc.tensor.matmul(
                            psc[:],
                            wpt[:, hh, :],
                            oTs[hh][:, t * QW : (t + 1) * QW],
                            start=(hh == 0),
                            stop=(hh == G - 1),
                        )
                    csb = csb_pool.tile([P, QW], BF16, tag="csb")
                    # DVE: C pieces weave into head 7's exp-saturated
                    # chunks, so this copy must stay off the ACT stream
                    with nc.allow_low_precision(reason="bf16 out"):
                        nc.vector.tensor_copy(csb[:], psc[:])
                    nc.sync.dma_start(
                        outT[m * P : (m + 1) * P, t * QW : (t + 1) * QW],
                        csb[:],
                    )

                def add_C_t(h, q, t):
                    # per t-chunk: 16 m pieces with the NEXT quad's wp load
                    # emitted before each quad's pieces (ring stays 1 ahead)
                    for mg in range(4):
                        if mg + 1 < 4:
                            add(h, q, emit_wp_loads, t, mg + 1)
                        elif t + 1 < NQ:
                            add(h, q, emit_wp_loads, t + 1, 0)
                        for m in range(mg * 4, mg * 4 + 4):
                            add(h, q, emit_C_m, t, m)

                # ---- PE filler schedule: slot (h, Q) -> list of thunks ----
                # v0 = v heads 0-3 (16 blocks), v1 = v heads 4-7 (16),
                # dqk m=12..15 = q/k heads 6-7, C = output projection.
                # Constraints: v0 tb in [4Q,4Q+3] before (0..3, Q); v1 tb
                # likewise before (4, Q); dqk m before head m//2; C t-chunk
                # after head 7's Q=t norm (deferred: emitted during Q=t+1).
                fill = {}

                def add(h, q, fn, *a):
                    fill.setdefault((h, q), []).append((fn, a))

                for tb in range(4, 8):
                    add(0, 0, emit_v_block, 0, tb)
                for tb in range(8, 12):
                    add(0, 1, emit_v_block, 0, tb)
                for tb in range(12, 16):
                    add(0, 2, emit_v_block, 0, tb)
                add(0, 3, emit_wv1_load)
                for tb in range(0, 3):
                    add(1, tb + 1, emit_v_block, 1, tb)
                for tb in range(3, 7):
                    add(2, tb - 3, emit_v_block, 1, tb)
                add(1, 3, emit_dqk_load, 12)
                add(2, 3, emit_dqk_load, 13)
                add(4, 3, emit_dqk_load, 14)
                add(5, 3, emit_dqk_load, 15)
                for i, c in enumerate(range(4)):     # m=12 in head 2/3
                    add(2 if i < 2 else 3, i % 2 * 2, emit_dqk_chunk, 12, c)
                for tb in range(7, 11):
                    add(3, tb - 7, emit_v_block, 1, tb)
                for c in range(4):                   # m=13 in head 3/4
                    add(3 if c < 2 else 4, c % 2, emit_dqk_chunk, 13, c)
                # (4,Q) consumes v1 tb<=4Q+3, so tb12-15 must precede (4,3)
                for tb, q in [(11, 0), (12, 0), (13, 1), (14, 1), (15, 2)]:
                    add(4, q, emit_v_block, 1, tb)
                for c in range(4):                   # m=14 in head 5
                    add(5, c, emit_dqk_chunk, 14, c)
                for c in range(4):                   # m=15 in head 6
                    add(6, c, emit_dqk_chunk, 15, c)
                add(6, 3, emit_wp_loads, 0, 0)
                add_C_t(7, 1, 0)
                add_C_t(7, 2, 1)
                add_C_t(7, 3, 2)

                # v0 tb0-3 must precede head 0 entirely
                for tb in range(4):
                    emit_v_block(0, tb)

                # q/k prefetch for heads 0,1 (heads 2+ prefetched inside the
                # loop).  Head 7's k never touches DRAM (m=15 writes its
                # SBUF tile directly), and its q prefetch is emitted after
                # m=14's qkd writes, so program order keeps RAW intact.
                qts, kts = {}, {}

                def prefetch_q(hp):
                    qts[hp] = q_pool.tile([P, T], BF16, tag="q",
                                          name=f"q{hp}")
                    nc.sync.dma_start(qts[hp][:], qkd[2 * hp])

                def prefetch_k(hp):
                    kts[hp] = k_pool.tile([P, T], BF16, tag="k",
                                          name=f"k{hp}")
                    nc.sync.dma_start(kts[hp][:], qkd[2 * hp + 1])

                def prefetch_qk(hp):
                    prefetch_q(hp)
                    if hp != 7:
                        prefetch_k(hp)

                prefetch_qk(0)
                prefetch_qk(1)

                def emit_attn_J(h, Q, J, qT, kT, acc_a, acc_b):
                    k_d = J - JPQ * Q  # diag idx if >= 0
                    co = max(k_d, 0) * P
                    pss = psS.tile([P, QW], F32, tag="pss")
                    nc.tensor.matmul(
                        pss[:, co:],
                        kT[:, J * P : (J + 1) * P],
                        qT[:, Q * QW + co : (Q + 1) * QW],
                        start=True,
                        stop=True,
                        skip_group_check=True,
                    )
                    pT = pt_pool.tile([P, QW], BF16, tag="pt")
                    nc.scalar.activation(
                        pT[:, co:], pss[:, co:],
                        mybir.ActivationFunctionType.Exp,
                        scale=scale,
                    )
                    if k_d >= 0:
                        with nc.allow_low_precision(reason="bf16 attn"):
                            nc.vector.tensor_mul(
                                pT[:, co : co + P],
                                pT[:, co : co + P],
                                mask_s[:],
                            )
                    # den chain split across DVE (even J) and Pool (odd J):
                    # either engine alone can't keep up with the exp cadence
                    # and a lagging chain holds PSUM ring slots, stalling PE
                    with nc.allow_low_precision(reason="f32 bits"):
                        if J == 0:
                            nc.vector.tensor_copy(acc_a[:], pT[:])
                        elif J == 1:
                            nc.gpsimd.tensor_copy(
                                acc_b[:, co:], pT[:, co:]
                            )
                        elif J % 2 == 0:
                            nc.vector.tensor_add(
                                acc_a[:, co:], acc_a[:, co:], pT[:, co:]
                            )
                        else:
                            nc.gpsimd.tensor_add(
                                acc_b[:, co:], acc_b[:, co:], pT[:, co:]
                            )
                    return co, pT

                def emit_chunk(h, Q, qT, kT, oT, den_pend, norm_pend,
                               emit_den, emit_norm):
                    jmax = JPQ * (Q + 1) - 1  # inclusive
                    # previous chunk's den reduction + norm run FIRST: the
                    # combine drained behind the previous chunk's work, and
                    # with the norm done up front, this slot's fillers
                    # (incl. phase C pieces at head 7) are legal to weave
                    # into the J-loop below.
                    emit_den()
                    if norm_pend:
                        emit_norm()
                    pso = psO.tile([P, QW], F32, tag="pso")
                    acc_a = acc_pool.tile([P, QW], BF16, tag="acca")
                    acc_b = accb_pool.tile([P, QW], BF16, tag="accb")
                    # PV(J) consumes exp(S(J)) from ACT; emit it LOOK
                    # S-matmuls later so PE never stalls on ACT.  The exp
                    # cadence (~535ns) exceeds the PE's per-J work (~426ns),
                    # so ACT-independent filler is woven in every few J's
                    # to keep the PE fed while ACT catches up.
                    LOOK = 3
                    fillers = fill.pop((h, Q), [])
                    pend = []
                    for J in range(jmax + 1 + LOOK):
                        if J <= jmax:
                            pend.append(
                                (J,)
                                + emit_attn_J(h, Q, J, qT, kT, acc_a, acc_b)
                            )
                        if J >= LOOK:
                            Jp, cop, pTp = pend.pop(0)
                            nc.tensor.matmul(
                                pso[:, cop:],
                                v_all[:, Jp, h * P : (h + 1) * P],
                                pTp[:, cop:],
                                start=(Jp == 0),
                                stop=(Jp == jmax),
                                skip_group_check=True,
                            )
                        if J % 4 == 3 and fillers:
                            fn, a = fillers.pop(0)
                            fn(*a)
                    # raw o^T out of PSUM (DVE: ACT is exp-saturated here)
                    with nc.allow_low_precision(reason="bf16 out"):
                        nc.vector.tensor_copy(
                            oT[:, Q * QW : (Q + 1) * QW], pso[:]
                        )
                    # den combine runs on DVE NOW (right behind this chunk's
                    # own adds, so it drains ASAP), but the dependent psden
                    # matmul is deferred one chunk (emit_den above) so the
                    # in-order PE never waits on the chain.
                    # acc_b is initialized at J=1, whose co is 128 for Q=0
                    # (columns below have no odd-J contributions)
                    cb = P if Q == 0 else 0
                    with nc.allow_low_precision(reason="f32 bits"):
                        nc.vector.tensor_add(
                            acc_a[:, cb:], acc_a[:, cb:], acc_b[:, cb:]
                        )
                    den_pend.append((Q, acc_a))
                    # remaining PE filler for this slot
                    for fn, a in fillers:
                        fn(*a)

                oTs = []

                def process_head(h):
                    qT = qts.pop(h)
                    kT = kts.pop(h)
                    oT = oacc_pool.tile([P, T], BF16, tag="oacc",
                                        name=f"oT{h}")
                    oTs.append(oT)

                    # deferred den-reduction / normalization state
                    den_pend = []
                    norm_pend = []

                    def emit_den(den_pend=den_pend, norm_pend=norm_pend):
                        if not den_pend:
                            return
                        Qp, acc_a = den_pend.pop(0)
                        psden = psM.tile([P, QW], F32, tag="psm")
                        nc.tensor.matmul(
                            psden[:], ones_s[:], acc_a[:],
                            start=True, stop=True, skip_group_check=True,
                        )
                        nc.scalar.activation(
                            psden[:], psden[:],
                            mybir.ActivationFunctionType.Ln,
                        )
                        rinv = rinv_pool.tile([P, QW], F32, tag="rinv")
                        nc.scalar.activation(
                            rinv[:], psden[:],
                            mybir.ActivationFunctionType.Exp,
                            scale=-1.0,
                        )
                        norm_pend.append((Qp, rinv))

                    def emit_norm(oT=oT, norm_pend=norm_pend):
                        Qp, rinv_p = norm_pend.pop(0)
                        with nc.allow_low_precision(reason="bf16 out"):
                            nc.vector.tensor_mul(
                                oT[:, Qp * QW : (Qp + 1) * QW],
                                oT[:, Qp * QW : (Qp + 1) * QW],
                                rinv_p[:],
                            )

                    for Q in range(NQ):
                        emit_chunk(h, Q, qT, kT, oT, den_pend, norm_pend,
                                   emit_den, emit_norm)
                    emit_den()
                    emit_norm()
                    if h + 2 < G:
                        prefetch_qk(h + 2)

                for h in range(G):
                    process_head(h)

                assert not fill, f"unemitted fillers: {list(fill)}"
                # final output projection chunk (needs head 7 fully normed)
                for mg in range(4):
                    if mg + 1 < 4:
                        emit_wp_loads(3, mg + 1)
                for m in range(KO):
                    emit_C_m(3, m)

    _split_multi_waits(nc)
    return nc


# --------------------------------------------------------------------------
def _prep_core_inputs(xb, w_attn, w_proj, rope_cos, rope_sin, g, G=8):
    """Host-side shard prep for one core: batch slice xb (T, C), group g."""
    T, C = xb.shape
    KO = C // P
    VN = 512
    NV = (G * P) // VN
    gc = g * G * P  # column offset of this group within one qkv section
    bf = ml_dtypes.bfloat16

    # x^T arranged [p, kc, t]
    xt = np.ascontiguousarray(
        xb.T.reshape(KO, P, T).transpose(1, 0, 2)
    ).astype(bf)

    # q,k columns for this group, RoPE pair-permuted (2i,2i+1) -> (i,64+i),
    # interleaved m: 2h = q head h, 2h+1 = k head h
    perm = np.empty(P, dtype=np.int64)
    perm[:64] = np.arange(0, P, 2)
    perm[64:] = np.arange(1, P, 2)
    wq = w_attn[:, gc : gc + G * P].reshape(C, G, P)[:, :, perm]
    wk = w_attn[:, C + gc : C + gc + G * P].reshape(C, G, P)[:, :, perm]
    wqk_i = np.empty((2 * G, C, P), dtype=np.float32)
    wqk_i[0::2] = wq.transpose(1, 0, 2)
    wqk_i[1::2] = wk.transpose(1, 0, 2)
    # [2G, C, P] -> [2G, P(part), KO, P(cols)]
    wqk = np.ascontiguousarray(
        wqk_i.reshape(2 * G, KO, P, P).transpose(0, 2, 1, 3)
    ).astype(bf)

    wv_cols = w_attn[:, 2 * C + gc : 2 * C + gc + G * P]  # (C, G*128)
    wv = np.ascontiguousarray(
        wv_cols.reshape(KO, P, NV, VN).transpose(2, 1, 0, 3)
    ).astype(bf)

    wp_rows = w_proj[gc : gc + G * P, :]  # (G*128, C)
    wp = np.ascontiguousarray(
        wp_rows.reshape(G, P, KO, P).transpose(2, 1, 0, 3)
    ).astype(bf)

    cT = rope_cos[:T].T  # (64, T)
    sT = rope_sin[:T].T
    cospT = np.ascontiguousarray(np.concatenate([cT, cT], axis=0)).astype(bf)
    sinpT = np.ascontiguousarray(np.concatenate([-sT, sT], axis=0)).astype(bf)
    mask = np.triu(np.ones((P, P), dtype=np.float32)).astype(bf)

    return {
        "xt": xt,
        "wqk": wqk,
        "wv": wv,
        "wp": wp,
        "cosp": cospT,
        "sinp": sinpT,
        "maskt": mask,
        "onesd": np.ones((P, P), dtype=np.float32).astype(bf),
    }


_NC_CACHE = {}
TRACE = False
LAST_RESULTS = None


def kernel(x, w_attn, w_proj, rope_cos, rope_sin):
    from concourse.bass_utils import run_bass_kernel_spmd

    x = np.asarray(x, dtype=np.float32)
    w_attn = np.asarray(w_attn, dtype=np.float32)
    w_proj = np.asarray(w_proj, dtype=np.float32)
    rope_cos = np.asarray(rope_cos, dtype=np.float32)
    rope_sin = np.asarray(rope_sin, dtype=np.float32)

    B, T, C = x.shape
    G = 8  # heads per group (16 heads / 2 groups)

    key = (T, C, G)
    if key not in _NC_CACHE:
        _NC_CACHE[key] = build_attention_core(T=T, C=C, G=G)
    nc = _NC_CACHE[key]

    in_maps = []
    for core in range(8):
        b, g = core // 2, core % 2
        in_maps.append(
            _prep_core_inputs(x[b], w_attn, w_proj, rope_cos, rope_sin, g, G=G)
        )

    res = run_bass_kernel_spmd(nc, in_maps, list(range(8)), trace=TRACE)
    global LAST_RESULTS
    LAST_RESULTS = res

    y = np.empty((B, T, C), dtype=np.float32)
    for b in range(B):
        acc = (
            res.results[2 * b]["outT"].astype(np.float32)
            + res.results[2 * b + 1]["outT"].astype(np.float32)
        )
        y[b] = acc.T
    return y
